# revision 29
# baseline (speedup 1.0000x reference)
"""Trainium2 Bass kernel for nn_Attention_9431748182241.

Module: x -> 1x1 qkv conv -> {3x3,5x5,7x7} depthwise convs -> q/k/v 1x1
projections -> per-head channel attention (CxC over L2-normalized q,k)
-> 1x1 out projection.

Algorithm: the entire pre-attention pipeline is linear in x and collapses
(host-side weight folding) to

    q = sum_{t in 7x7 offsets} Bq_t @ S_t(x)        (same for k, v)

where S_t is the zero-padded spatial shift. On-device, per 4-image-row
spatial tile (512 cols):
  - q, k: 24 DoubleRow fp8(e4m3) matmuls (2 taps per instruction, paired
    along dx/dy so the pair is one extra AP dim with constant stride) + 1
    plain fp8 matmul, accumulated in PSUM. fp8 noise is harmless here: q,k
    only feed L2-normalized Gram correlations averaged over 16384-long
    dots (end-to-end sim: 4.9e-4 rel err).
  - v: 49 fp32r matmuls (output-critical path needs full precision).
Norms (ACT square+accum) and the per-head Gram matrix (PE transpose +
matmul, PSUM-accumulated across all tiles) are computed inline; v streams
to DRAM. A tiny finale builds softmax attention per head, folds it with
W_out into a single [128,128] matrix, and a second pass produces
out = (W_out A) @ v.

Sharding: data-parallel -- batch 8 across 8 cores, identical program (SPMD),
no collectives.
"""

from contextlib import ExitStack

import ml_dtypes
import numpy as np

import concourse.bass as bass
import concourse.bacc as bacc
import concourse.mybir as mybir
import concourse.tile as tile
from concourse.bass_utils import run_bass_kernel_spmd

B, C, H, W = 8, 128, 128, 128
HEADS = 8
DH = C // HEADS  # 16
PAD = 3
NOFF = 49  # 7x7 offset union
TILE_ROWS = 4
GRAM_MODE = "pe_bf16"  # dma_bf16 | pe_bf16 | dve
f32 = mybir.dt.float32
f32r = mybir.dt.float32r
bf16 = mybir.dt.bfloat16
fp8 = mybir.dt.float8e4
DR = mybir.MatmulPerfMode.DoubleRow

_NC_CACHE = {}

# Tap order for the fp8 DoubleRow path: pairs are adjacent in this list.
# HW requires an EVEN k-pair stride (odd deltas fault the exec unit), so:
# 21 dx-pairs (delta=2), 3 dy-pairs at dx=2 (delta=2*wp), single (2,2) last.
TAP_ORDER = []
for _dy in range(-3, 4):
    for _dx0 in (-3, -2, 1):
        TAP_ORDER += [(_dy, _dx0), (_dy, _dx0 + 2)]
for _dy0 in (-3, -2, 1):
    TAP_ORDER += [(_dy0, 2), (_dy0 + 2, 2)]
TAP_ORDER.append((2, 2))
assert len(TAP_ORDER) == NOFF and len(set(TAP_ORDER)) == NOFF

SLACK = 8  # fp8 x tail slack (last q,k tile's widest tap overshoots by 3)

# out-conv tap split: inner 5x5 stays a folded dense conv on the PE; the 24
# outer-ring taps (7x7 only) run as factorized per-channel MACs on DVE/GpSimd
# over z_v = V_v x, re-entering the PE accumulation via (M W_v3) @ R matmuls.
OFFSETS = [(dy, dx) for dy in range(-3, 4) for dx in range(-3, 4)]
INNER = [t for t in OFFSETS if max(abs(t[0]), abs(t[1])) <= 2]
RING = [t for t in OFFSETS if max(abs(t[0]), abs(t[1])) == 3]
assert len(INNER) == 25 and len(RING) == 24
FRONT = 4  # fp8 x front slack (tile 0 garbage pad columns read 3 before start)


def fold_weights(w_qkv, w_dw3, w_dw5, w_dw7, w_q, w_k, w_v):
    """[3, 49, C, C] f64: out_o = sum_t B[o,t] @ S_t(x). Tap index = offsets
    order (dy,dx) row-major."""
    w_qkv = np.asarray(w_qkv, np.float64)
    dws = [np.asarray(w, np.float64) for w in (w_dw3, w_dw5, w_dw7)]
    w_o = [np.asarray(w, np.float64) for w in (w_q, w_k, w_v)]

    Bm = np.zeros((3, NOFF, C, C))
    offsets = [(dy, dx) for dy in range(-3, 4) for dx in range(-3, 4)]
    for o in range(3):
        part = o * C
        V = w_qkv[part : part + C, :]
        for ti, (dy, dx) in enumerate(offsets):
            A = np.zeros((C, C))
            for g, k in enumerate((3, 5, 7)):
                p = k // 2
                if abs(dy) <= p and abs(dx) <= p:
                    taps = dws[g][part : part + C, 0, dy + p, dx + p]
                    A += w_o[o][:, g * C : (g + 1) * C] * taps[None, :]
            Bm[o, ti] = A @ V
    return Bm


def build_nc(h=H, w=W, dbg=False):
    """Build the per-core Bass program. h, w: image dims (w must be 128)."""
    assert w == 128 and h % TILE_ROWS == 0
    hw = h * w
    nt = h // TILE_ROWS
    N = TILE_ROWS * w  # moving-dim per tile
    hp, wp = h + 2 * PAD, w + 2 * PAD

    # q,k fp8 path tiles the PADDED coordinate space in 3-image-row spans so
    # every tap read is one contiguous slice (DoubleRow rhs must be 3D).
    QK_ROWS = 3
    qk_rows = [QK_ROWS] * (h // QK_ROWS) + ([h % QK_ROWS] if h % QK_ROWS else [])
    nt_qk = len(qk_rows)

    nc = bacc.Bacc("TRN2", target_bir_lowering=False, debug=False)
    dbg_d = {}
    if dbg:
        for nm, shp, dt_ in [
            ("dq", [C, hw], bf16), ("dk", [C, hw], bf16), ("dg", [C, C], f32),
            ("dabd", [C, C], f32), ("dmf", [C, C], f32), ("dnq", [C, 1], f32),
        ]:
            dbg_d[nm] = nc.dram_tensor(nm, shp, dt_, kind="ExternalOutput")
    x_d = nc.dram_tensor("x", [C, hp * wp], f32, kind="ExternalInput")
    x8_d = nc.dram_tensor("x8", [C, FRONT + hp * wp + SLACK], fp8, kind="ExternalInput")
    # v-conv fold operands (f32, non-transposed): 25 inner Bv_t blocks then
    # W_v3 (block 25) for the ring path
    wB_d = nc.dram_tensor("wB", [C, NOFF * C], f32, kind="ExternalInput")
    # q,k-conv weights (fp8), TAP_ORDER order, o in {0=q,1=k}
    wb8_d = nc.dram_tensor("wb8", [C, 2 * NOFF * C], fp8, kind="ExternalInput")
    woutT_d = nc.dram_tensor("woutT", [C, C], f32, kind="ExternalInput")
    tempc_d = nc.dram_tensor("tempc", [C, 1], f32, kind="ExternalInput")
    ident_d = nc.dram_tensor("ident", [C, C], f32, kind="ExternalInput")
    maskn_d = nc.dram_tensor("maskn", [C, C], f32, kind="ExternalInput")
    zcon_d = nc.dram_tensor("zcon", [C, C], f32, kind="ExternalInput")  # ones
    y_d = nc.dram_tensor("y", [C, hw], f32, kind="ExternalOutput")

    with tile.TileContext(nc) as tc, ExitStack() as ctx:
        sb_x = ctx.enter_context(tc.tile_pool(name="sb_x", bufs=1))
        sb_x8 = ctx.enter_context(tc.tile_pool(name="sb_x8", bufs=1))
        sb_w = ctx.enter_context(tc.tile_pool(name="sb_w", bufs=1))
        sb_w8 = ctx.enter_context(tc.tile_pool(name="sb_w8", bufs=1))
        sb_c = ctx.enter_context(tc.tile_pool(name="sb_c", bufs=1))
        sb_qk = ctx.enter_context(tc.tile_pool(name="sb_qk", bufs=4))
        sb_qkT = ctx.enter_context(tc.tile_pool(name="sb_qkT", bufs=4))
        sb_sq = ctx.enter_context(tc.tile_pool(name="sb_sq", bufs=1))
        sb_n = ctx.enter_context(tc.tile_pool(name="sb_n", bufs=1))
        sb_f = ctx.enter_context(tc.tile_pool(name="sb_f", bufs=1))
        sb_wf = ctx.enter_context(tc.tile_pool(name="sb_wf", bufs=1))
        sb_o = ctx.enter_context(tc.tile_pool(name="sb_o", bufs=3))
        ps_qk = ctx.enter_context(tc.tile_pool(name="ps_qk", bufs=4, space="PSUM"))
        ps_out = ctx.enter_context(tc.tile_pool(name="ps_out", bufs=1, space="PSUM"))
        ps_tr = ctx.enter_context(tc.tile_pool(name="ps_tr", bufs=2, space="PSUM"))
        ps_g = ctx.enter_context(tc.tile_pool(name="ps_g", bufs=1, space="PSUM"))

        # ---- constants / inputs into SBUF ----
        ident = sb_c.tile([C, C], f32, tag="ident")
        nc.sync.dma_start(ident[:], ident_d.ap())
        ident_b = sb_c.tile([C, C], bf16, tag="ident_b")
        nc.vector.tensor_copy(ident_b[:], ident[:])

        # fp8 x first (q,k tiles start on it), chunked so tile 0 starts ASAP
        xp8 = sb_x8.tile([C, FRONT + hp * wp + SLACK], fp8)
        x8_len = FRONT + hp * wp + SLACK
        bnd8 = [0, FRONT + 10 * wp, FRONT + 24 * wp] + [
            x8_len * c // 4 for c in range(1, 5)
        ]
        bnd8 = sorted(set(min(4 * ((b + 3) // 4), x8_len) for b in bnd8))
        u32 = mybir.dt.uint32
        for c0 in range(len(bnd8) - 1):
            nc.sync.dma_start(
                xp8[:, bnd8[c0] : bnd8[c0 + 1]]
                .rearrange("p (a b) -> p a b", b=4)
                .bitcast(u32),
                x8_d.ap()[:, bnd8[c0] : bnd8[c0 + 1]]
                .rearrange("p (a b) -> p a b", b=4)
                .bitcast(u32),
            )
        # f32 x (v path) on the same queue after
        xp = sb_x.tile([C, hp * wp], f32)
        bnd = [0, 10 * wp, 24 * wp] + [hp * wp * c // 6 for c in range(1, 7)]
        bnd = sorted(set(min(b, hp * wp) for b in bnd))
        for c0 in range(len(bnd) - 1):
            nc.sync.dma_start(
                xp[:, bnd[c0] : bnd[c0 + 1]].bitcast(f32r),
                x_d.ap()[:, bnd[c0] : bnd[c0 + 1]].bitcast(f32r),
            )
        xp3 = xp[:].rearrange("p (a b) -> p a b", b=wp)
        # weights on the other queue; fp8 q,k weights first
        wb8 = sb_w8.tile([C, 2 * NOFF * C], fp8)
        w8bnd = [0, 3 * C, 12 * C] + [2 * NOFF * C * c // 4 for c in range(1, 5)]
        w8bnd = sorted(set(4 * ((b + 3) // 4) for b in w8bnd))
        for c0 in range(len(w8bnd) - 1):
            nc.scalar.dma_start(
                wb8[:, w8bnd[c0] : w8bnd[c0 + 1]]
                .rearrange("p (a b) -> p a b", b=4)
                .bitcast(u32),
                wb8_d.ap()[:, w8bnd[c0] : w8bnd[c0 + 1]]
                .rearrange("p (a b) -> p a b", b=4)
                .bitcast(u32),
            )
        wb8_3d = wb8[:].rearrange("p (t c) -> p t c", c=C)
        wB = sb_w.tile([C, NOFF * C], f32)
        wbnd = [NOFF * C * c // 6 for c in range(7)]
        wbnd = sorted(set(wbnd))
        for c0 in range(len(wbnd) - 1):
            nc.scalar.dma_start(
                wB[:, wbnd[c0] : wbnd[c0 + 1]].bitcast(f32r),
                wB_d.ap()[:, wbnd[c0] : wbnd[c0 + 1]].bitcast(f32r),
            )
        zcon = sb_c.tile([C, C], f32, tag="zcon")
        nc.sync.dma_start(zcon[:], zcon_d.ap())
        ones1 = zcon[0:1, 0:C]
        woutT = sb_c.tile([C, C], f32, tag="woutT")
        nc.sync.dma_start(woutT[:], woutT_d.ap())
        tempc = sb_c.tile([C, 1], f32, tag="tempc")
        nc.sync.dma_start(tempc[:], tempc_d.ap())
        maskn = sb_c.tile([C, C], f32, tag="maskn")
        nc.sync.dma_start(maskn[:], maskn_d.ap())

        nq_p = sb_n.tile([C, nt_qk], f32, tag="nq_p")
        nk_p = sb_n.tile([C, nt_qk], f32, tag="nk_p")

        g_ps = ps_g.tile([C, C], f32)

        offsets = [(dy, dx) for dy in range(-3, 4) for dx in range(-3, 4)]

        xp8_ap = xp8[:]
        part_stride = xp8_ap.ap[0][0]

        def qk_mms_fp8(o, out_ps, i, nspan):
            """24 DoubleRow pairs + 1 single fp8 matmul for output o (0=q,1=k)
            over the padded-coordinate span of q,k tile i (contiguous reads);
            output columns at pad positions are garbage and get compacted
            away on the PSUM->SBUF copy."""
            s_i = FRONT + (PAD + i * QK_ROWS) * wp
            for pk in range(24):
                dy0, dx0 = TAP_ORDER[2 * pk]
                dy1, dx1 = TAP_ORDER[2 * pk + 1]
                delta = (dy1 - dy0) * wp + (dx1 - dx0)
                off = s_i + dy0 * wp + dx0
                rhs = bass.AP(
                    xp8_ap.tensor,
                    xp8_ap.offset + off,
                    [[part_stride, C], [delta, 2], [1, nspan]],
                )
                nc.tensor.matmul(
                    out_ps,
                    wb8_3d[:, o * NOFF + 2 * pk : o * NOFF + 2 * pk + 2, :],
                    rhs,
                    start=(pk == 0),
                    stop=False,
                    perf_mode=DR,
                    tile_position=(0, 0),
                )
            # leftover single tap (3,3), plain fp8 matmul
            dy, dx = TAP_ORDER[48]
            off = s_i + dy * wp + dx
            rhs = bass.AP(
                xp8_ap.tensor,
                xp8_ap.offset + off,
                [[part_stride, C], [1, nspan]],
            )
            nc.tensor.matmul(
                out_ps,
                wb8_3d[:, o * NOFF + 48, :],
                rhs,
                start=False,
                stop=True,
                tile_position=(0, 0),
            )

        def out_mms(out_ps, w_fold, i):
            """49 accumulating fp32r matmuls with A-folded weights: the tile
            output IS the final out = (W_out A Bv) * x."""
            y0 = i * TILE_ROWS
            for ti, (dy, dx) in enumerate(OFFSETS):
                rhs = xp3[
                    :,
                    y0 + PAD + dy : y0 + PAD + dy + TILE_ROWS,
                    PAD + dx : PAD + dx + w,
                ]
                nc.tensor.matmul(
                    out_ps[:],
                    w_fold[:, bass.ts(ti, C)].bitcast(f32r),
                    rhs.bitcast(f32r),
                    start=(ti == 0),
                    stop=(ti == NOFF - 1),
                )

        # ---- phase 1: q/k (fp8 DoubleRow, padded-coord tiles) + v (fp32r,
        # row-aligned tiles), norms, Gram. The two loops are interleaved as
        # priority hints; the Tile scheduler orders by dependencies.
        NSP = QK_ROWS * wp  # max padded span
        NQK = QK_ROWS * w  # max valid columns per q,k tile

        def qk_tile(i):
            rows = qk_rows[i]
            nspan = rows * wp
            nv = rows * w
            q_ps = ps_qk.tile([C, NSP], f32, tag="qk_ps")
            qk_mms_fp8(0, q_ps[:, :nspan], i, nspan)
            k_ps = ps_qk.tile([C, NSP], f32, tag="qk_ps")
            qk_mms_fp8(1, k_ps[:, :nspan], i, nspan)

            # compact valid columns (pad positions dropped) to bf16
            q_s = sb_qk.tile([C, NQK], bf16, tag="qk")
            nc.vector.tensor_copy(
                q_s[:, :nv].rearrange("p (r c) -> p r c", c=w),
                q_ps[:].rearrange("p (r c) -> p r c", c=wp)[
                    :, :rows, PAD : PAD + w
                ],
            )
            k_s = sb_qk.tile([C, NQK], bf16, tag="qk")
            nc.vector.tensor_copy(
                k_s[:, :nv].rearrange("p (r c) -> p r c", c=w),
                k_ps[:].rearrange("p (r c) -> p r c", c=wp)[
                    :, :rows, PAD : PAD + w
                ],
            )

            # norms: ACT square with accumulate on the compacted bf16 copies
            sq_q = sb_sq.tile([C, NQK], f32, tag="sq")
            nc.scalar.activation(
                sq_q[:, :nv], q_s[:, :nv], mybir.ActivationFunctionType.Square,
                accum_out=nq_p[:, i : i + 1],
            )
            sq_k = sb_sq.tile([C, NQK], f32, tag="sq")
            nc.scalar.activation(
                sq_k[:, :nv], k_s[:, :nv], mybir.ActivationFunctionType.Square,
                accum_out=nk_p[:, i : i + 1],
            )

            if dbg:
                nc.sync.dma_start(
                    dbg_d["dq"].ap()[:, i * NQK : i * NQK + nv], q_s[:, :nv]
                )
                nc.sync.dma_start(
                    dbg_d["dk"].ap()[:, i * NQK : i * NQK + nv], k_s[:, :nv]
                )

            # transpose 128-chunks; Gram accumulates G += q_chunk @ k_chunk^T
            qT = sb_qkT.tile([C, NQK], bf16, tag="qkT")
            kT = sb_qkT.tile([C, NQK], bf16, tag="qkT")
            for j in range(rows):
                t_ps = ps_tr.tile([C, C], bf16, tag="tr")
                nc.tensor.transpose(
                    t_ps[:], q_s[:, bass.ts(j, C)], ident_b[:]
                )
                nc.vector.tensor_copy(qT[:, bass.ts(j, C)], t_ps[:])
                t_ps2 = ps_tr.tile([C, C], bf16, tag="tr")
                nc.tensor.transpose(
                    t_ps2[:], k_s[:, bass.ts(j, C)], ident_b[:]
                )
                nc.vector.tensor_copy(kT[:, bass.ts(j, C)], t_ps2[:])
            for j in range(rows):
                nc.tensor.matmul(
                    g_ps[:],
                    qT[:, bass.ts(j, C)],
                    kT[:, bass.ts(j, C)],
                    start=(i == 0 and j == 0),
                    stop=(i == nt_qk - 1 and j == rows - 1),
                )

        for i in range(nt_qk):
            qk_tile(i)

        # ---- finale: softmax attention + fold with W_out ----
        nq = sb_f.tile([C, 1], f32, tag="nq")
        nc.vector.reduce_sum(nq[:], nq_p[:], axis=mybir.AxisListType.X)
        nk = sb_f.tile([C, 1], f32, tag="nk")
        nc.vector.reduce_sum(nk[:], nk_p[:], axis=mybir.AxisListType.X)
        # 1/||q|| = reciprocal(sqrt(sum q^2)); norms >> eps=1e-12 here
        nq_s = sb_f.tile([C, 1], f32, tag="nq_s")
        nc.scalar.sqrt(nq_s[:], nq[:])
        rq = sb_f.tile([C, 1], f32, tag="rq")
        nc.vector.reciprocal(rq[:], nq_s[:])
        nk_s = sb_f.tile([C, 1], f32, tag="nk_s")
        nc.scalar.sqrt(nk_s[:], nk[:])
        rk = sb_f.tile([C, 1], f32, tag="rk")
        nc.vector.reciprocal(rk[:], nk_s[:])
        # rq2 = rq * temperature(per-channel)
        rq2 = sb_f.tile([C, 1], f32, tag="rq2")
        nc.vector.tensor_mul(rq2[:], rq[:], tempc[:])

        # rk as a row, broadcast down partitions via outer product with ones
        rk_row_ps = ps_tr.tile([C, C], f32, tag="tr")
        nc.tensor.transpose(rk_row_ps[0:1, :], rk[:], ident[:])
        rk_row = sb_f.tile([1, C], f32, tag="rk_row")
        nc.vector.tensor_copy(rk_row[:], rk_row_ps[0:1, :])
        rkb_ps = ps_tr.tile([C, C], f32, tag="tr")
        nc.tensor.matmul(rkb_ps[:], ones1[:], rk_row[:], start=True, stop=True)
        rkb = sb_f.tile([C, C], f32, tag="rkb")
        nc.vector.tensor_copy(rkb[:], rkb_ps[:])

        # masked softmax over the full [C, C] Gram: off-head-block entries
        # get a -1e4 bias -> exp underflows to exactly 0, so the softmax
        # result IS the block-diagonal attention matrix A.
        g_s = sb_f.tile([C, C], f32, tag="g_s")
        nc.vector.tensor_copy(g_s[:], g_ps[:])
        g1 = sb_f.tile([C, C], f32, tag="g1")
        nc.vector.tensor_mul(g1[:], g_s[:], rkb[:])
        g2 = sb_f.tile([C, C], f32, tag="g2")
        nc.vector.tensor_scalar_mul(g2[:], g1[:], rq2[:])
        g3 = sb_f.tile([C, C], f32, tag="g3")
        nc.vector.tensor_add(g3[:], g2[:], maskn[:])
        mx = sb_f.tile([C, 1], f32, tag="mx")
        nc.vector.reduce_max(mx[:], g3[:], axis=mybir.AxisListType.X)
        nmx = sb_f.tile([C, 1], f32, tag="nmx")
        nc.vector.tensor_scalar_mul(nmx[:], mx[:], -1.0)
        ex = sb_f.tile([C, C], f32, tag="ex")
        ssum = sb_f.tile([C, 1], f32, tag="ssum")
        nc.scalar.activation(
            ex[:], g3[:], mybir.ActivationFunctionType.Exp,
            bias=nmx[:], accum_out=ssum[:],
        )
        rs = sb_f.tile([C, 1], f32, tag="rs")
        nc.vector.reciprocal(rs[:], ssum[:])
        a_bd = sb_f.tile([C, C], f32, tag="a_bd")
        nc.vector.tensor_scalar_mul(a_bd[:], ex[:], rs[:])

        if dbg:
            nc.sync.dma_start(dbg_d["dg"].ap(), g_s[:])
            nc.sync.dma_start(dbg_d["dabd"].ap(), a_bd[:])
            nc.sync.dma_start(dbg_d["dnq"].ap(), nq[:])

        # M_final = (W_out A)^T = A^T W_out^T
        mf_ps = ps_tr.tile([C, C], f32, tag="tr")
        nc.tensor.matmul(mf_ps[:], a_bd[:], woutT[:], start=True, stop=True)
        m_final = sb_f.tile([C, C], f32, tag="m_final")
        nc.vector.tensor_copy(m_final[:].bitcast(f32r), mf_ps[:])
        if dbg:
            nc.sync.dma_start(dbg_d["dmf"].ap(), m_final[:])

        # ---- phase 2: fold M = W_out A into the v-conv weights, then the
        # out conv produces y directly. wB holds NON-transposed Bv_t blocks,
        # so matmul(lhsT=Bv_t, rhs=M^T) = Bv_t^T M^T = (M Bv_t)^T = conv lhsT.
        w_fold = sb_wf.tile([C, NOFF * C], f32, tag="w_fold")
        for ti in range(NOFF):
            wf_ps = ps_tr.tile([C, C], f32, tag="tr")
            nc.tensor.matmul(
                wf_ps[:],
                wB[:, bass.ts(ti, C)].bitcast(f32r),
                m_final[:].bitcast(f32r),
                start=True,
                stop=True,
            )
            nc.vector.tensor_copy(
                w_fold[:, bass.ts(ti, C)].bitcast(f32r), wf_ps[:]
            )

        for i in range(nt):
            o_ps = ps_out.tile([C, N], f32, tag="out_ps")
            out_mms(o_ps, w_fold, i)
            o_s = sb_o.tile([C, N], f32, tag="o_s")
            nc.vector.tensor_copy(o_s[:], o_ps[:])
            nc.sync.dma_start(y_d.ap()[:, bass.ts(i, N)], o_s[:])

    nc.compile()
    return nc


def _pow2_scale(m, cap=224.0):
    return 2.0 ** np.floor(np.log2(cap / max(np.abs(m).max(), 1e-30)))


def _prep_inputs(inputs, h=H, w=W):
    Bm = fold_weights(
        inputs["w_qkv"], inputs["w_dw3"], inputs["w_dw5"], inputs["w_dw7"],
        inputs["w_q"], inputs["w_k"], inputs["w_v"],
    )
    offsets = [(dy, dx) for dy in range(-3, 4) for dx in range(-3, 4)]
    off_idx = {t: i for i, t in enumerate(offsets)}
    # v-conv fold operands: block t = B[2,t] (NOT transposed; the on-device
    # fold matmul(lhsT=Bv_t, rhs=M^T) produces the conv lhsT (M Bv_t)^T)
    wB = np.concatenate(
        [np.ascontiguousarray(Bm[2, ti], dtype=np.float32) for ti in range(NOFF)],
        axis=1,
    )
    # q,k fp8 weights in TAP_ORDER, scaled per-output (scale cancels in the
    # normalized attention, so no compensation is needed downstream)
    wb8_blocks = []
    for o in (0, 1):
        s = _pow2_scale(Bm[o])
        for dy, dx in TAP_ORDER:
            blk = (Bm[o, off_idx[(dy, dx)]].T * s).astype(np.float32)
            wb8_blocks.append(blk.astype(ml_dtypes.float8_e4m3))
    wb8 = np.concatenate(wb8_blocks, axis=1)  # [C, 2*49*C] fp8

    woutT = np.ascontiguousarray(np.asarray(inputs["w_out"]).T).astype(np.float32)
    tempc = np.repeat(
        np.asarray(inputs["temperature"], np.float32).reshape(HEADS), DH
    ).reshape(C, 1)
    ident = np.eye(C, dtype=np.float32)
    maskn = np.full((C, C), -1e4, np.float32)
    for hd in range(HEADS):
        maskn[hd * DH : (hd + 1) * DH, hd * DH : (hd + 1) * DH] = 0.0
    zcon = np.ones((C, C), np.float32)
    x = np.asarray(inputs["x"], np.float32)
    nb = x.shape[0]
    hp, wp = h + 2 * PAD, w + 2 * PAD
    xpad = np.zeros((nb, C, hp, wp), np.float32)
    xpad[:, :, PAD : PAD + h, PAD : PAD + w] = x.reshape(nb, C, h, w)
    xpad8 = np.zeros((nb, C, FRONT + hp * wp + SLACK), np.float32)
    xpad8[:, :, FRONT : FRONT + hp * wp] = xpad.reshape(nb, C, hp * wp)
    xpad8 = xpad8.astype(ml_dtypes.float8_e4m3)
    in_maps = [
        {
            "x": np.ascontiguousarray(xpad[b].reshape(C, hp * wp)),
            "x8": np.ascontiguousarray(xpad8[b]),
            "wB": wB,
            "wb8": wb8,
            "woutT": woutT,
            "tempc": tempc,
            "ident": ident,
            "maskn": maskn,
            "zcon": zcon,
        }
        for b in range(nb)
    ]
    return in_maps


def kernel(**inputs):
    if "nc" not in _NC_CACHE:
        _NC_CACHE["nc"] = build_nc()
    nc = _NC_CACHE["nc"]
    in_maps = _prep_inputs(inputs)
    res = run_bass_kernel_spmd(nc, in_maps, core_ids=list(range(B)))
    out = np.stack([res.results[b]["y"].reshape(C, H, W) for b in range(B)])
    return out.astype(np.float32)


# revision 30
# speedup vs baseline: 1.0414x; 1.0414x over previous
"""Trainium2 Bass kernel for nn_Attention_9431748182241.

Module: x -> 1x1 qkv conv -> {3x3,5x5,7x7} depthwise convs -> q/k/v 1x1
projections -> per-head channel attention (CxC over L2-normalized q,k)
-> 1x1 out projection.

Algorithm: the entire pre-attention pipeline is linear in x and collapses
(host-side weight folding) to

    q = sum_{t in 7x7 offsets} Bq_t @ S_t(x)        (same for k, v)

where S_t is the zero-padded spatial shift. On-device, per 4-image-row
spatial tile (512 cols):
  - q, k: 24 DoubleRow fp8(e4m3) matmuls (2 taps per instruction, paired
    along dx/dy so the pair is one extra AP dim with constant stride) + 1
    plain fp8 matmul, accumulated in PSUM. fp8 noise is harmless here: q,k
    only feed L2-normalized Gram correlations averaged over 16384-long
    dots (end-to-end sim: 4.9e-4 rel err).
  - v: 49 fp32r matmuls (output-critical path needs full precision).
Norms (ACT square+accum) and the per-head Gram matrix (PE transpose +
matmul, PSUM-accumulated across all tiles) are computed inline; v streams
to DRAM. A tiny finale builds softmax attention per head, folds it with
W_out into a single [128,128] matrix, and a second pass produces
out = (W_out A) @ v.

Sharding: data-parallel -- batch 8 across 8 cores, identical program (SPMD),
no collectives.
"""

from contextlib import ExitStack

import ml_dtypes
import numpy as np

import concourse.bass as bass
import concourse.bacc as bacc
import concourse.mybir as mybir
import concourse.tile as tile
from concourse.bass_utils import run_bass_kernel_spmd

B, C, H, W = 8, 128, 128, 128
HEADS = 8
DH = C // HEADS  # 16
PAD = 3
NOFF = 49  # 7x7 offset union
TILE_ROWS = 4
GRAM_MODE = "pe_bf16"  # dma_bf16 | pe_bf16 | dve
f32 = mybir.dt.float32
f32r = mybir.dt.float32r
bf16 = mybir.dt.bfloat16
fp8 = mybir.dt.float8e4
DR = mybir.MatmulPerfMode.DoubleRow

_NC_CACHE = {}

# Tap order for the fp8 DoubleRow path: pairs are adjacent in this list.
# HW requires an EVEN k-pair stride (odd deltas fault the exec unit), so:
# 21 dx-pairs (delta=2), 3 dy-pairs at dx=2 (delta=2*wp), single (2,2) last.
TAP_ORDER = []
for _dy in range(-3, 4):
    for _dx0 in (-3, -2, 1):
        TAP_ORDER += [(_dy, _dx0), (_dy, _dx0 + 2)]
for _dy0 in (-3, -2, 1):
    TAP_ORDER += [(_dy0, 2), (_dy0 + 2, 2)]
TAP_ORDER.append((2, 2))
assert len(TAP_ORDER) == NOFF and len(set(TAP_ORDER)) == NOFF

SLACK = 8  # fp8 x tail slack (last q,k tile's widest tap overshoots by 3)

# out-conv tap split: inner 5x5 stays a folded dense conv on the PE; the 24
# outer-ring taps (7x7 only) run as factorized per-channel MACs on DVE/GpSimd
# over z_v = V_v x, re-entering the PE accumulation via (M W_v3) @ R matmuls.
OFFSETS = [(dy, dx) for dy in range(-3, 4) for dx in range(-3, 4)]
INNER = [t for t in OFFSETS if max(abs(t[0]), abs(t[1])) <= 2]
RING = [t for t in OFFSETS if max(abs(t[0]), abs(t[1])) == 3]
assert len(INNER) == 25 and len(RING) == 24
FRONT = 4  # fp8 x front slack (tile 0 garbage pad columns read 3 before start)


def fold_weights(w_qkv, w_dw3, w_dw5, w_dw7, w_q, w_k, w_v):
    """[3, 49, C, C] f64: out_o = sum_t B[o,t] @ S_t(x). Tap index = offsets
    order (dy,dx) row-major."""
    w_qkv = np.asarray(w_qkv, np.float64)
    dws = [np.asarray(w, np.float64) for w in (w_dw3, w_dw5, w_dw7)]
    w_o = [np.asarray(w, np.float64) for w in (w_q, w_k, w_v)]

    Bm = np.zeros((3, NOFF, C, C))
    offsets = [(dy, dx) for dy in range(-3, 4) for dx in range(-3, 4)]
    for o in range(3):
        part = o * C
        V = w_qkv[part : part + C, :]
        for ti, (dy, dx) in enumerate(offsets):
            A = np.zeros((C, C))
            for g, k in enumerate((3, 5, 7)):
                p = k // 2
                if abs(dy) <= p and abs(dx) <= p:
                    taps = dws[g][part : part + C, 0, dy + p, dx + p]
                    A += w_o[o][:, g * C : (g + 1) * C] * taps[None, :]
            Bm[o, ti] = A @ V
    return Bm


def build_nc(h=H, w=W, dbg=False):
    """Build the per-core Bass program. h, w: image dims (w must be 128)."""
    assert w == 128 and h % TILE_ROWS == 0
    hw = h * w
    nt = h // TILE_ROWS
    N = TILE_ROWS * w  # moving-dim per tile
    hp, wp = h + 2 * PAD, w + 2 * PAD

    # q,k fp8 path tiles the PADDED coordinate space in 3-image-row spans so
    # every tap read is one contiguous slice (DoubleRow rhs must be 3D).
    QK_ROWS = 3
    qk_rows = [QK_ROWS] * (h // QK_ROWS) + ([h % QK_ROWS] if h % QK_ROWS else [])
    nt_qk = len(qk_rows)

    nc = bacc.Bacc("TRN2", target_bir_lowering=False, debug=False)
    dbg_d = {}
    if dbg:
        for nm, shp, dt_ in [
            ("dq", [C, hw], bf16), ("dk", [C, hw], bf16), ("dg", [C, C], f32),
            ("dabd", [C, C], f32), ("dmf", [C, C], f32), ("dnq", [C, 1], f32),
        ]:
            dbg_d[nm] = nc.dram_tensor(nm, shp, dt_, kind="ExternalOutput")
    x_d = nc.dram_tensor("x", [C, hp * wp], f32, kind="ExternalInput")
    x8_d = nc.dram_tensor("x8", [C, FRONT + hp * wp + SLACK], fp8, kind="ExternalInput")
    # v-conv fold operands (f32, non-transposed): 25 inner Bv_t blocks then
    # W_v3 (block 25) for the ring path
    wB_d = nc.dram_tensor("wB", [C, NOFF * C], f32, kind="ExternalInput")
    # q,k-conv weights (fp8), TAP_ORDER order, o in {0=q,1=k}
    wb8_d = nc.dram_tensor("wb8", [C, 2 * NOFF * C], fp8, kind="ExternalInput")
    woutT_d = nc.dram_tensor("woutT", [C, C], f32, kind="ExternalInput")
    tempc_d = nc.dram_tensor("tempc", [C, 1], f32, kind="ExternalInput")
    ident_d = nc.dram_tensor("ident", [C, C], f32, kind="ExternalInput")
    maskn_d = nc.dram_tensor("maskn", [C, C], f32, kind="ExternalInput")
    zcon_d = nc.dram_tensor("zcon", [C, C], f32, kind="ExternalInput")  # ones
    y_d = nc.dram_tensor("y", [C, hw], f32, kind="ExternalOutput")

    with tile.TileContext(nc) as tc, ExitStack() as ctx:
        sb_x = ctx.enter_context(tc.tile_pool(name="sb_x", bufs=1))
        sb_x8 = ctx.enter_context(tc.tile_pool(name="sb_x8", bufs=1))
        sb_w = ctx.enter_context(tc.tile_pool(name="sb_w", bufs=1))
        sb_w8 = ctx.enter_context(tc.tile_pool(name="sb_w8", bufs=1))
        sb_c = ctx.enter_context(tc.tile_pool(name="sb_c", bufs=1))
        sb_qk = ctx.enter_context(tc.tile_pool(name="sb_qk", bufs=4))
        sb_qkT = ctx.enter_context(tc.tile_pool(name="sb_qkT", bufs=4))
        sb_sq = ctx.enter_context(tc.tile_pool(name="sb_sq", bufs=1))
        sb_n = ctx.enter_context(tc.tile_pool(name="sb_n", bufs=1))
        sb_f = ctx.enter_context(tc.tile_pool(name="sb_f", bufs=1))
        sb_wf = ctx.enter_context(tc.tile_pool(name="sb_wf", bufs=1))
        sb_o = ctx.enter_context(tc.tile_pool(name="sb_o", bufs=3))
        ps_qk = ctx.enter_context(tc.tile_pool(name="ps_qk", bufs=3, space="PSUM"))
        ps_out = ctx.enter_context(tc.tile_pool(name="ps_out", bufs=2, space="PSUM"))
        ps_tr = ctx.enter_context(tc.tile_pool(name="ps_tr", bufs=2, space="PSUM"))
        ps_g = ctx.enter_context(tc.tile_pool(name="ps_g", bufs=1, space="PSUM"))

        # ---- constants / inputs into SBUF ----
        ident = sb_c.tile([C, C], f32, tag="ident")
        nc.sync.dma_start(ident[:], ident_d.ap())
        ident_b = sb_c.tile([C, C], bf16, tag="ident_b")
        nc.vector.tensor_copy(ident_b[:], ident[:])

        # fp8 x first (q,k tiles start on it), chunked so tile 0 starts ASAP
        xp8 = sb_x8.tile([C, FRONT + hp * wp + SLACK], fp8)
        x8_len = FRONT + hp * wp + SLACK
        bnd8 = [0, FRONT + 10 * wp, FRONT + 24 * wp] + [
            x8_len * c // 4 for c in range(1, 5)
        ]
        bnd8 = sorted(set(min(4 * ((b + 3) // 4), x8_len) for b in bnd8))
        u32 = mybir.dt.uint32
        for c0 in range(len(bnd8) - 1):
            nc.sync.dma_start(
                xp8[:, bnd8[c0] : bnd8[c0 + 1]]
                .rearrange("p (a b) -> p a b", b=4)
                .bitcast(u32),
                x8_d.ap()[:, bnd8[c0] : bnd8[c0 + 1]]
                .rearrange("p (a b) -> p a b", b=4)
                .bitcast(u32),
            )
        # f32 x (v path) on the same queue after
        xp = sb_x.tile([C, hp * wp], f32)
        bnd = [0, 10 * wp, 24 * wp] + [hp * wp * c // 6 for c in range(1, 7)]
        bnd = sorted(set(min(b, hp * wp) for b in bnd))
        for c0 in range(len(bnd) - 1):
            nc.sync.dma_start(
                xp[:, bnd[c0] : bnd[c0 + 1]].bitcast(f32r),
                x_d.ap()[:, bnd[c0] : bnd[c0 + 1]].bitcast(f32r),
            )
        xp3 = xp[:].rearrange("p (a b) -> p a b", b=wp)
        # weights on the other queue; fp8 q,k weights first
        wb8 = sb_w8.tile([C, 2 * NOFF * C], fp8)
        w8bnd = [0, 3 * C, 12 * C] + [2 * NOFF * C * c // 4 for c in range(1, 5)]
        w8bnd = sorted(set(4 * ((b + 3) // 4) for b in w8bnd))
        for c0 in range(len(w8bnd) - 1):
            nc.scalar.dma_start(
                wb8[:, w8bnd[c0] : w8bnd[c0 + 1]]
                .rearrange("p (a b) -> p a b", b=4)
                .bitcast(u32),
                wb8_d.ap()[:, w8bnd[c0] : w8bnd[c0 + 1]]
                .rearrange("p (a b) -> p a b", b=4)
                .bitcast(u32),
            )
        wb8_3d = wb8[:].rearrange("p (t c) -> p t c", c=C)
        wB = sb_w.tile([C, NOFF * C], f32)
        wbnd = [NOFF * C * c // 6 for c in range(7)]
        wbnd = sorted(set(wbnd))
        for c0 in range(len(wbnd) - 1):
            nc.scalar.dma_start(
                wB[:, wbnd[c0] : wbnd[c0 + 1]].bitcast(f32r),
                wB_d.ap()[:, wbnd[c0] : wbnd[c0 + 1]].bitcast(f32r),
            )
        zcon = sb_c.tile([C, C], f32, tag="zcon")
        nc.sync.dma_start(zcon[:], zcon_d.ap())
        ones1 = zcon[0:1, 0:C]
        woutT = sb_c.tile([C, C], f32, tag="woutT")
        nc.sync.dma_start(woutT[:], woutT_d.ap())
        tempc = sb_c.tile([C, 1], f32, tag="tempc")
        nc.sync.dma_start(tempc[:], tempc_d.ap())
        maskn = sb_c.tile([C, C], f32, tag="maskn")
        nc.sync.dma_start(maskn[:], maskn_d.ap())

        nq_p = sb_n.tile([C, nt_qk], f32, tag="nq_p")
        nk_p = sb_n.tile([C, nt_qk], f32, tag="nk_p")

        g_ps = ps_g.tile([C, C], f32)

        offsets = [(dy, dx) for dy in range(-3, 4) for dx in range(-3, 4)]

        xp8_ap = xp8[:]
        part_stride = xp8_ap.ap[0][0]

        def qk_mms_fp8(o, out_ps, i, nspan):
            """24 DoubleRow pairs + 1 single fp8 matmul for output o (0=q,1=k)
            over the padded-coordinate span of q,k tile i (contiguous reads);
            output columns at pad positions are garbage and get compacted
            away on the PSUM->SBUF copy."""
            s_i = FRONT + (PAD + i * QK_ROWS) * wp
            for pk in range(24):
                dy0, dx0 = TAP_ORDER[2 * pk]
                dy1, dx1 = TAP_ORDER[2 * pk + 1]
                delta = (dy1 - dy0) * wp + (dx1 - dx0)
                off = s_i + dy0 * wp + dx0
                rhs = bass.AP(
                    xp8_ap.tensor,
                    xp8_ap.offset + off,
                    [[part_stride, C], [delta, 2], [1, nspan]],
                )
                nc.tensor.matmul(
                    out_ps,
                    wb8_3d[:, o * NOFF + 2 * pk : o * NOFF + 2 * pk + 2, :],
                    rhs,
                    start=(pk == 0),
                    stop=False,
                    perf_mode=DR,
                    tile_position=(0, 0),
                )
            # leftover single tap (3,3), plain fp8 matmul
            dy, dx = TAP_ORDER[48]
            off = s_i + dy * wp + dx
            rhs = bass.AP(
                xp8_ap.tensor,
                xp8_ap.offset + off,
                [[part_stride, C], [1, nspan]],
            )
            nc.tensor.matmul(
                out_ps,
                wb8_3d[:, o * NOFF + 48, :],
                rhs,
                start=False,
                stop=True,
                tile_position=(0, 0),
            )

        def out_mms(out_ps, w_fold, i):
            """49 accumulating fp32r matmuls with A-folded weights: the tile
            output IS the final out = (W_out A Bv) * x."""
            y0 = i * TILE_ROWS
            for ti, (dy, dx) in enumerate(OFFSETS):
                rhs = xp3[
                    :,
                    y0 + PAD + dy : y0 + PAD + dy + TILE_ROWS,
                    PAD + dx : PAD + dx + w,
                ]
                nc.tensor.matmul(
                    out_ps[:],
                    w_fold[:, bass.ts(ti, C)].bitcast(f32r),
                    rhs.bitcast(f32r),
                    start=(ti == 0),
                    stop=(ti == NOFF - 1),
                )

        # ---- phase 1: q/k (fp8 DoubleRow, padded-coord tiles) + v (fp32r,
        # row-aligned tiles), norms, Gram. The two loops are interleaved as
        # priority hints; the Tile scheduler orders by dependencies.
        NSP = QK_ROWS * wp  # max padded span
        NQK = QK_ROWS * w  # max valid columns per q,k tile

        def qk_tile(i):
            rows = qk_rows[i]
            nspan = rows * wp
            nv = rows * w
            q_ps = ps_qk.tile([C, NSP], f32, tag="qk_ps")
            qk_mms_fp8(0, q_ps[:, :nspan], i, nspan)
            k_ps = ps_qk.tile([C, NSP], f32, tag="qk_ps")
            qk_mms_fp8(1, k_ps[:, :nspan], i, nspan)

            # compact valid columns (pad positions dropped) to bf16
            q_s = sb_qk.tile([C, NQK], bf16, tag="qk")
            nc.vector.tensor_copy(
                q_s[:, :nv].rearrange("p (r c) -> p r c", c=w),
                q_ps[:].rearrange("p (r c) -> p r c", c=wp)[
                    :, :rows, PAD : PAD + w
                ],
            )
            k_s = sb_qk.tile([C, NQK], bf16, tag="qk")
            nc.vector.tensor_copy(
                k_s[:, :nv].rearrange("p (r c) -> p r c", c=w),
                k_ps[:].rearrange("p (r c) -> p r c", c=wp)[
                    :, :rows, PAD : PAD + w
                ],
            )

            # norms: ACT square with accumulate on the compacted bf16 copies
            sq_q = sb_sq.tile([C, NQK], f32, tag="sq")
            nc.scalar.activation(
                sq_q[:, :nv], q_s[:, :nv], mybir.ActivationFunctionType.Square,
                accum_out=nq_p[:, i : i + 1],
            )
            sq_k = sb_sq.tile([C, NQK], f32, tag="sq")
            nc.scalar.activation(
                sq_k[:, :nv], k_s[:, :nv], mybir.ActivationFunctionType.Square,
                accum_out=nk_p[:, i : i + 1],
            )

            if dbg:
                nc.sync.dma_start(
                    dbg_d["dq"].ap()[:, i * NQK : i * NQK + nv], q_s[:, :nv]
                )
                nc.sync.dma_start(
                    dbg_d["dk"].ap()[:, i * NQK : i * NQK + nv], k_s[:, :nv]
                )

            # transpose 128-chunks; Gram accumulates G += q_chunk @ k_chunk^T
            qT = sb_qkT.tile([C, NQK], bf16, tag="qkT")
            kT = sb_qkT.tile([C, NQK], bf16, tag="qkT")
            for j in range(rows):
                t_ps = ps_tr.tile([C, C], bf16, tag="tr")
                nc.tensor.transpose(
                    t_ps[:], q_s[:, bass.ts(j, C)], ident_b[:]
                )
                nc.vector.tensor_copy(qT[:, bass.ts(j, C)], t_ps[:])
                t_ps2 = ps_tr.tile([C, C], bf16, tag="tr")
                nc.tensor.transpose(
                    t_ps2[:], k_s[:, bass.ts(j, C)], ident_b[:]
                )
                nc.vector.tensor_copy(kT[:, bass.ts(j, C)], t_ps2[:])
            for j in range(rows):
                nc.tensor.matmul(
                    g_ps[:],
                    qT[:, bass.ts(j, C)],
                    kT[:, bass.ts(j, C)],
                    start=(i == 0 and j == 0),
                    stop=(i == nt_qk - 1 and j == rows - 1),
                )

        for i in range(nt_qk):
            qk_tile(i)

        # ---- finale: softmax attention + fold with W_out ----
        nq = sb_f.tile([C, 1], f32, tag="nq")
        nc.vector.reduce_sum(nq[:], nq_p[:], axis=mybir.AxisListType.X)
        nk = sb_f.tile([C, 1], f32, tag="nk")
        nc.vector.reduce_sum(nk[:], nk_p[:], axis=mybir.AxisListType.X)
        # 1/||q|| = reciprocal(sqrt(sum q^2)); norms >> eps=1e-12 here
        nq_s = sb_f.tile([C, 1], f32, tag="nq_s")
        nc.scalar.sqrt(nq_s[:], nq[:])
        rq = sb_f.tile([C, 1], f32, tag="rq")
        nc.vector.reciprocal(rq[:], nq_s[:])
        nk_s = sb_f.tile([C, 1], f32, tag="nk_s")
        nc.scalar.sqrt(nk_s[:], nk[:])
        rk = sb_f.tile([C, 1], f32, tag="rk")
        nc.vector.reciprocal(rk[:], nk_s[:])
        # rq2 = rq * temperature(per-channel)
        rq2 = sb_f.tile([C, 1], f32, tag="rq2")
        nc.vector.tensor_mul(rq2[:], rq[:], tempc[:])

        # rk as a row, broadcast down partitions via outer product with ones
        rk_row_ps = ps_tr.tile([C, C], f32, tag="tr")
        nc.tensor.transpose(rk_row_ps[0:1, :], rk[:], ident[:])
        rk_row = sb_f.tile([1, C], f32, tag="rk_row")
        nc.vector.tensor_copy(rk_row[:], rk_row_ps[0:1, :])
        rkb_ps = ps_tr.tile([C, C], f32, tag="tr")
        nc.tensor.matmul(rkb_ps[:], ones1[:], rk_row[:], start=True, stop=True)
        rkb = sb_f.tile([C, C], f32, tag="rkb")
        nc.vector.tensor_copy(rkb[:], rkb_ps[:])

        # masked softmax over the full [C, C] Gram: off-head-block entries
        # get a -1e4 bias -> exp underflows to exactly 0, so the softmax
        # result IS the block-diagonal attention matrix A.
        g_s = sb_f.tile([C, C], f32, tag="g_s")
        nc.vector.tensor_copy(g_s[:], g_ps[:])
        g1 = sb_f.tile([C, C], f32, tag="g1")
        nc.vector.tensor_mul(g1[:], g_s[:], rkb[:])
        g2 = sb_f.tile([C, C], f32, tag="g2")
        nc.vector.tensor_scalar_mul(g2[:], g1[:], rq2[:])
        g3 = sb_f.tile([C, C], f32, tag="g3")
        nc.vector.tensor_add(g3[:], g2[:], maskn[:])
        mx = sb_f.tile([C, 1], f32, tag="mx")
        nc.vector.reduce_max(mx[:], g3[:], axis=mybir.AxisListType.X)
        nmx = sb_f.tile([C, 1], f32, tag="nmx")
        nc.vector.tensor_scalar_mul(nmx[:], mx[:], -1.0)
        ex = sb_f.tile([C, C], f32, tag="ex")
        ssum = sb_f.tile([C, 1], f32, tag="ssum")
        nc.scalar.activation(
            ex[:], g3[:], mybir.ActivationFunctionType.Exp,
            bias=nmx[:], accum_out=ssum[:],
        )
        rs = sb_f.tile([C, 1], f32, tag="rs")
        nc.vector.reciprocal(rs[:], ssum[:])
        a_bd = sb_f.tile([C, C], f32, tag="a_bd")
        nc.vector.tensor_scalar_mul(a_bd[:], ex[:], rs[:])

        if dbg:
            nc.sync.dma_start(dbg_d["dg"].ap(), g_s[:])
            nc.sync.dma_start(dbg_d["dabd"].ap(), a_bd[:])
            nc.sync.dma_start(dbg_d["dnq"].ap(), nq[:])

        # M_final = (W_out A)^T = A^T W_out^T
        mf_ps = ps_tr.tile([C, C], f32, tag="tr")
        nc.tensor.matmul(mf_ps[:], a_bd[:], woutT[:], start=True, stop=True)
        m_final = sb_f.tile([C, C], f32, tag="m_final")
        nc.vector.tensor_copy(m_final[:].bitcast(f32r), mf_ps[:])
        if dbg:
            nc.sync.dma_start(dbg_d["dmf"].ap(), m_final[:])

        # ---- phase 2: fold M = W_out A into the v-conv weights, then the
        # out conv produces y directly. wB holds NON-transposed Bv_t blocks,
        # so matmul(lhsT=Bv_t, rhs=M^T) = Bv_t^T M^T = (M Bv_t)^T = conv lhsT.
        w_fold = sb_wf.tile([C, NOFF * C], f32, tag="w_fold")
        for ti in range(NOFF):
            wf_ps = ps_tr.tile([C, C], f32, tag="tr")
            nc.tensor.matmul(
                wf_ps[:],
                wB[:, bass.ts(ti, C)].bitcast(f32r),
                m_final[:].bitcast(f32r),
                start=True,
                stop=True,
            )
            nc.vector.tensor_copy(
                w_fold[:, bass.ts(ti, C)].bitcast(f32r), wf_ps[:]
            )

        for i in range(nt):
            o_ps = ps_out.tile([C, N], f32, tag="out_ps")
            out_mms(o_ps, w_fold, i)
            o_s = sb_o.tile([C, N], f32, tag="o_s")
            nc.vector.tensor_copy(o_s[:], o_ps[:])
            nc.sync.dma_start(y_d.ap()[:, bass.ts(i, N)], o_s[:])

    nc.compile()
    return nc


def _pow2_scale(m, cap=224.0):
    return 2.0 ** np.floor(np.log2(cap / max(np.abs(m).max(), 1e-30)))


def _prep_inputs(inputs, h=H, w=W):
    Bm = fold_weights(
        inputs["w_qkv"], inputs["w_dw3"], inputs["w_dw5"], inputs["w_dw7"],
        inputs["w_q"], inputs["w_k"], inputs["w_v"],
    )
    offsets = [(dy, dx) for dy in range(-3, 4) for dx in range(-3, 4)]
    off_idx = {t: i for i, t in enumerate(offsets)}
    # v-conv fold operands: block t = B[2,t] (NOT transposed; the on-device
    # fold matmul(lhsT=Bv_t, rhs=M^T) produces the conv lhsT (M Bv_t)^T)
    wB = np.concatenate(
        [np.ascontiguousarray(Bm[2, ti], dtype=np.float32) for ti in range(NOFF)],
        axis=1,
    )
    # q,k fp8 weights in TAP_ORDER, scaled per-output (scale cancels in the
    # normalized attention, so no compensation is needed downstream)
    wb8_blocks = []
    for o in (0, 1):
        s = _pow2_scale(Bm[o])
        for dy, dx in TAP_ORDER:
            blk = (Bm[o, off_idx[(dy, dx)]].T * s).astype(np.float32)
            wb8_blocks.append(blk.astype(ml_dtypes.float8_e4m3))
    wb8 = np.concatenate(wb8_blocks, axis=1)  # [C, 2*49*C] fp8

    woutT = np.ascontiguousarray(np.asarray(inputs["w_out"]).T).astype(np.float32)
    tempc = np.repeat(
        np.asarray(inputs["temperature"], np.float32).reshape(HEADS), DH
    ).reshape(C, 1)
    ident = np.eye(C, dtype=np.float32)
    maskn = np.full((C, C), -1e4, np.float32)
    for hd in range(HEADS):
        maskn[hd * DH : (hd + 1) * DH, hd * DH : (hd + 1) * DH] = 0.0
    zcon = np.ones((C, C), np.float32)
    x = np.asarray(inputs["x"], np.float32)
    nb = x.shape[0]
    hp, wp = h + 2 * PAD, w + 2 * PAD
    xpad = np.zeros((nb, C, hp, wp), np.float32)
    xpad[:, :, PAD : PAD + h, PAD : PAD + w] = x.reshape(nb, C, h, w)
    xpad8 = np.zeros((nb, C, FRONT + hp * wp + SLACK), np.float32)
    xpad8[:, :, FRONT : FRONT + hp * wp] = xpad.reshape(nb, C, hp * wp)
    xpad8 = xpad8.astype(ml_dtypes.float8_e4m3)
    in_maps = [
        {
            "x": np.ascontiguousarray(xpad[b].reshape(C, hp * wp)),
            "x8": np.ascontiguousarray(xpad8[b]),
            "wB": wB,
            "wb8": wb8,
            "woutT": woutT,
            "tempc": tempc,
            "ident": ident,
            "maskn": maskn,
            "zcon": zcon,
        }
        for b in range(nb)
    ]
    return in_maps


def kernel(**inputs):
    if "nc" not in _NC_CACHE:
        _NC_CACHE["nc"] = build_nc()
    nc = _NC_CACHE["nc"]
    in_maps = _prep_inputs(inputs)
    res = run_bass_kernel_spmd(nc, in_maps, core_ids=list(range(B)))
    out = np.stack([res.results[b]["y"].reshape(C, H, W) for b in range(B)])
    return out.astype(np.float32)


# revision 32
# speedup vs baseline: 1.2935x; 1.2421x over previous
"""Trainium2 Bass kernel for nn_Attention_9431748182241.

Module: x -> 1x1 qkv conv -> {3x3,5x5,7x7} depthwise convs -> q/k/v 1x1
projections -> per-head channel attention (CxC over L2-normalized q,k)
-> 1x1 out projection.

Algorithm: the entire pre-attention pipeline is linear in x and collapses
(host-side weight folding) to

    q = sum_{t in 7x7 offsets} Bq_t @ S_t(x)        (same for k, v)

where S_t is the zero-padded spatial shift. On-device, per 4-image-row
spatial tile (512 cols):
  - q, k: 24 DoubleRow fp8(e4m3) matmuls (2 taps per instruction, paired
    along dx/dy so the pair is one extra AP dim with constant stride) + 1
    plain fp8 matmul, accumulated in PSUM. fp8 noise is harmless here: q,k
    only feed L2-normalized Gram correlations averaged over 16384-long
    dots (end-to-end sim: 4.9e-4 rel err).
  - v: 49 fp32r matmuls (output-critical path needs full precision).
Norms (ACT square+accum) and the per-head Gram matrix (PE transpose +
matmul, PSUM-accumulated across all tiles) are computed inline; v streams
to DRAM. A tiny finale builds softmax attention per head, folds it with
W_out into a single [128,128] matrix, and a second pass produces
out = (W_out A) @ v.

Sharding: data-parallel -- batch 8 across 8 cores, identical program (SPMD),
no collectives.
"""

from contextlib import ExitStack

import ml_dtypes
import numpy as np

import concourse.bass as bass
import concourse.bacc as bacc
import concourse.mybir as mybir
import concourse.tile as tile
from concourse.bass_utils import run_bass_kernel_spmd

B, C, H, W = 8, 128, 128, 128
HEADS = 8
DH = C // HEADS  # 16
PAD = 3
NOFF = 49  # 7x7 offset union
TILE_ROWS = 4
GRAM_MODE = "pe_bf16"  # dma_bf16 | pe_bf16 | dve
f32 = mybir.dt.float32
f32r = mybir.dt.float32r
bf16 = mybir.dt.bfloat16
fp8 = mybir.dt.float8e4
DR = mybir.MatmulPerfMode.DoubleRow

_NC_CACHE = {}

# Tap order for the fp8 DoubleRow path: pairs are adjacent in this list.
# HW requires an EVEN k-pair stride (odd deltas fault the exec unit), so:
# 21 dx-pairs (delta=2), 3 dy-pairs at dx=2 (delta=2*wp), single (2,2) last.
TAP_ORDER = []
for _dy in range(-3, 4):
    for _dx0 in (-3, -2, 1):
        TAP_ORDER += [(_dy, _dx0), (_dy, _dx0 + 2)]
for _dy0 in (-3, -2, 1):
    TAP_ORDER += [(_dy0, 2), (_dy0 + 2, 2)]
TAP_ORDER.append((2, 2))
assert len(TAP_ORDER) == NOFF and len(set(TAP_ORDER)) == NOFF

SLACK = 8  # fp8 x tail slack (last q,k tile's widest tap overshoots by 3)

# out-conv tap split: inner 5x5 stays a folded dense conv on the PE; the 24
# outer-ring taps (7x7 only) run as factorized per-channel MACs on DVE/GpSimd
# over z_v = V_v x, re-entering the PE accumulation via (M W_v3) @ R matmuls.
OFFSETS = [(dy, dx) for dy in range(-3, 4) for dx in range(-3, 4)]
INNER = [t for t in OFFSETS if max(abs(t[0]), abs(t[1])) <= 2]
RING = [t for t in OFFSETS if max(abs(t[0]), abs(t[1])) == 3]
assert len(INNER) == 25 and len(RING) == 24
FRONT = 4  # fp8 x front slack (tile 0 garbage pad columns read 3 before start)


def fold_weights(w_qkv, w_dw3, w_dw5, w_dw7, w_q, w_k, w_v):
    """[3, 49, C, C] f64: out_o = sum_t B[o,t] @ S_t(x). Tap index = offsets
    order (dy,dx) row-major."""
    w_qkv = np.asarray(w_qkv, np.float64)
    dws = [np.asarray(w, np.float64) for w in (w_dw3, w_dw5, w_dw7)]
    w_o = [np.asarray(w, np.float64) for w in (w_q, w_k, w_v)]

    Bm = np.zeros((3, NOFF, C, C))
    offsets = [(dy, dx) for dy in range(-3, 4) for dx in range(-3, 4)]
    for o in range(3):
        part = o * C
        V = w_qkv[part : part + C, :]
        for ti, (dy, dx) in enumerate(offsets):
            A = np.zeros((C, C))
            for g, k in enumerate((3, 5, 7)):
                p = k // 2
                if abs(dy) <= p and abs(dx) <= p:
                    taps = dws[g][part : part + C, 0, dy + p, dx + p]
                    A += w_o[o][:, g * C : (g + 1) * C] * taps[None, :]
            Bm[o, ti] = A @ V
    return Bm


def build_nc(h=H, w=W, dbg=False):
    """Build the per-core Bass program. h, w: image dims (w must be 128)."""
    assert w == 128 and h % TILE_ROWS == 0
    hw = h * w
    nt = h // TILE_ROWS
    N = TILE_ROWS * w  # moving-dim per tile
    hp, wp = h + 2 * PAD, w + 2 * PAD

    # q,k fp8 path tiles the PADDED coordinate space in 3-image-row spans so
    # every tap read is one contiguous slice (DoubleRow rhs must be 3D).
    QK_ROWS = 3
    qk_rows = [QK_ROWS] * (h // QK_ROWS) + ([h % QK_ROWS] if h % QK_ROWS else [])
    nt_qk = len(qk_rows)

    nc = bacc.Bacc("TRN2", target_bir_lowering=False, debug=False)
    dbg_d = {}
    if dbg:
        for nm, shp, dt_ in [
            ("dq", [C, hw], bf16), ("dk", [C, hw], bf16), ("dg", [C, C], f32),
            ("dabd", [C, C], f32), ("dmf", [C, C], f32), ("dnq", [C, 1], f32),
        ]:
            dbg_d[nm] = nc.dram_tensor(nm, shp, dt_, kind="ExternalOutput")
    x_d = nc.dram_tensor("x", [C, hp * wp], bf16, kind="ExternalInput")
    x8_d = nc.dram_tensor("x8", [C, FRONT + hp * wp + SLACK], fp8, kind="ExternalInput")
    # v-conv fold operands (f32, non-transposed): 25 inner Bv_t blocks then
    # W_v3 (block 25) for the ring path
    wB_d = nc.dram_tensor("wB", [C, NOFF * C], f32, kind="ExternalInput")
    # q,k-conv weights (fp8), TAP_ORDER order, o in {0=q,1=k}
    wb8_d = nc.dram_tensor("wb8", [C, 2 * NOFF * C], fp8, kind="ExternalInput")
    woutT_d = nc.dram_tensor("woutT", [C, C], f32, kind="ExternalInput")
    tempc_d = nc.dram_tensor("tempc", [C, 1], f32, kind="ExternalInput")
    ident_d = nc.dram_tensor("ident", [C, C], f32, kind="ExternalInput")
    maskn_d = nc.dram_tensor("maskn", [C, C], f32, kind="ExternalInput")
    zcon_d = nc.dram_tensor("zcon", [C, C], f32, kind="ExternalInput")  # ones
    y_d = nc.dram_tensor("y", [C, hw], f32, kind="ExternalOutput")

    with tile.TileContext(nc) as tc, ExitStack() as ctx:
        sb_x = ctx.enter_context(tc.tile_pool(name="sb_x", bufs=1))
        sb_x8 = ctx.enter_context(tc.tile_pool(name="sb_x8", bufs=1))
        sb_w = ctx.enter_context(tc.tile_pool(name="sb_w", bufs=1))
        sb_w8 = ctx.enter_context(tc.tile_pool(name="sb_w8", bufs=1))
        sb_c = ctx.enter_context(tc.tile_pool(name="sb_c", bufs=1))
        sb_qk = ctx.enter_context(tc.tile_pool(name="sb_qk", bufs=4))
        sb_qkT = ctx.enter_context(tc.tile_pool(name="sb_qkT", bufs=4))
        sb_sq = ctx.enter_context(tc.tile_pool(name="sb_sq", bufs=1))
        sb_n = ctx.enter_context(tc.tile_pool(name="sb_n", bufs=1))
        sb_f = ctx.enter_context(tc.tile_pool(name="sb_f", bufs=1))
        sb_wf = ctx.enter_context(tc.tile_pool(name="sb_wf", bufs=1))
        sb_o = ctx.enter_context(tc.tile_pool(name="sb_o", bufs=3))
        ps_qk = ctx.enter_context(tc.tile_pool(name="ps_qk", bufs=3, space="PSUM"))
        ps_out = ctx.enter_context(tc.tile_pool(name="ps_out", bufs=2, space="PSUM"))
        ps_tr = ctx.enter_context(tc.tile_pool(name="ps_tr", bufs=2, space="PSUM"))
        ps_g = ctx.enter_context(tc.tile_pool(name="ps_g", bufs=1, space="PSUM"))

        # ---- constants / inputs into SBUF ----
        ident = sb_c.tile([C, C], f32, tag="ident")
        nc.sync.dma_start(ident[:], ident_d.ap())
        ident_b = sb_c.tile([C, C], bf16, tag="ident_b")
        nc.vector.tensor_copy(ident_b[:], ident[:])

        u32 = mybir.dt.uint32
        # fp8 x first (q,k tiles start on it), chunked so tile 0 starts ASAP
        xp8 = sb_x8.tile([C, FRONT + hp * wp + SLACK], fp8)
        x8_len = FRONT + hp * wp + SLACK
        bnd8 = [0, FRONT + 10 * wp, FRONT + 24 * wp] + [
            x8_len * c // 4 for c in range(1, 5)
        ]
        bnd8 = sorted(set(min(4 * ((b + 3) // 4), x8_len) for b in bnd8))
        for c0 in range(len(bnd8) - 1):
            nc.sync.dma_start(
                xp8[:, bnd8[c0] : bnd8[c0 + 1]]
                .rearrange("p (a b) -> p a b", b=4)
                .bitcast(u32),
                x8_d.ap()[:, bnd8[c0] : bnd8[c0 + 1]]
                .rearrange("p (a b) -> p a b", b=4)
                .bitcast(u32),
            )
        # bf16 x (out-conv path) on the same queue after
        xp = sb_x.tile([C, hp * wp], bf16)
        bnd = [0, 10 * wp, 24 * wp] + [hp * wp * c // 6 for c in range(1, 7)]
        bnd = sorted(set(min(2 * ((b + 1) // 2), hp * wp) for b in bnd))
        for c0 in range(len(bnd) - 1):
            nc.sync.dma_start(
                xp[:, bnd[c0] : bnd[c0 + 1]]
                .rearrange("p (a b) -> p a b", b=2)
                .bitcast(u32),
                x_d.ap()[:, bnd[c0] : bnd[c0 + 1]]
                .rearrange("p (a b) -> p a b", b=2)
                .bitcast(u32),
            )
        xp3 = xp[:].rearrange("p (a b) -> p a b", b=wp)
        # weights on the other queue; fp8 q,k weights first
        wb8 = sb_w8.tile([C, 2 * NOFF * C], fp8)
        w8bnd = [0, 3 * C, 12 * C] + [2 * NOFF * C * c // 4 for c in range(1, 5)]
        w8bnd = sorted(set(4 * ((b + 3) // 4) for b in w8bnd))
        for c0 in range(len(w8bnd) - 1):
            nc.scalar.dma_start(
                wb8[:, w8bnd[c0] : w8bnd[c0 + 1]]
                .rearrange("p (a b) -> p a b", b=4)
                .bitcast(u32),
                wb8_d.ap()[:, w8bnd[c0] : w8bnd[c0 + 1]]
                .rearrange("p (a b) -> p a b", b=4)
                .bitcast(u32),
            )
        wb8_3d = wb8[:].rearrange("p (t c) -> p t c", c=C)
        wB = sb_w.tile([C, NOFF * C], f32)
        wbnd = [NOFF * C * c // 6 for c in range(7)]
        wbnd = sorted(set(wbnd))
        for c0 in range(len(wbnd) - 1):
            nc.scalar.dma_start(
                wB[:, wbnd[c0] : wbnd[c0 + 1]].bitcast(f32r),
                wB_d.ap()[:, wbnd[c0] : wbnd[c0 + 1]].bitcast(f32r),
            )
        zcon = sb_c.tile([C, C], f32, tag="zcon")
        nc.sync.dma_start(zcon[:], zcon_d.ap())
        ones1 = zcon[0:1, 0:C]
        woutT = sb_c.tile([C, C], f32, tag="woutT")
        nc.sync.dma_start(woutT[:], woutT_d.ap())
        tempc = sb_c.tile([C, 1], f32, tag="tempc")
        nc.sync.dma_start(tempc[:], tempc_d.ap())
        maskn = sb_c.tile([C, C], f32, tag="maskn")
        nc.sync.dma_start(maskn[:], maskn_d.ap())

        nq_p = sb_n.tile([C, nt_qk], f32, tag="nq_p")
        nk_p = sb_n.tile([C, nt_qk], f32, tag="nk_p")

        g_ps = ps_g.tile([C, C], f32)

        offsets = [(dy, dx) for dy in range(-3, 4) for dx in range(-3, 4)]

        xp8_ap = xp8[:]
        part_stride = xp8_ap.ap[0][0]

        def qk_mms_fp8(o, out_ps, i, nspan):
            """24 DoubleRow pairs + 1 single fp8 matmul for output o (0=q,1=k)
            over the padded-coordinate span of q,k tile i (contiguous reads);
            output columns at pad positions are garbage and get compacted
            away on the PSUM->SBUF copy."""
            s_i = FRONT + (PAD + i * QK_ROWS) * wp
            for pk in range(24):
                dy0, dx0 = TAP_ORDER[2 * pk]
                dy1, dx1 = TAP_ORDER[2 * pk + 1]
                delta = (dy1 - dy0) * wp + (dx1 - dx0)
                off = s_i + dy0 * wp + dx0
                rhs = bass.AP(
                    xp8_ap.tensor,
                    xp8_ap.offset + off,
                    [[part_stride, C], [delta, 2], [1, nspan]],
                )
                nc.tensor.matmul(
                    out_ps,
                    wb8_3d[:, o * NOFF + 2 * pk : o * NOFF + 2 * pk + 2, :],
                    rhs,
                    start=(pk == 0),
                    stop=False,
                    perf_mode=DR,
                    tile_position=(0, 0),
                )
            # leftover single tap (3,3), plain fp8 matmul
            dy, dx = TAP_ORDER[48]
            off = s_i + dy * wp + dx
            rhs = bass.AP(
                xp8_ap.tensor,
                xp8_ap.offset + off,
                [[part_stride, C], [1, nspan]],
            )
            nc.tensor.matmul(
                out_ps,
                wb8_3d[:, o * NOFF + 48, :],
                rhs,
                start=False,
                stop=True,
                tile_position=(0, 0),
            )

        def out_mms(out_ps, w_fold, i):
            """49 accumulating fp32r matmuls with A-folded weights: the tile
            output IS the final out = (W_out A Bv) * x."""
            y0 = i * TILE_ROWS
            for ti, (dy, dx) in enumerate(OFFSETS):
                rhs = xp3[
                    :,
                    y0 + PAD + dy : y0 + PAD + dy + TILE_ROWS,
                    PAD + dx : PAD + dx + w,
                ]
                nc.tensor.matmul(
                    out_ps[:],
                    w_fold[:, bass.ts(ti, C)],
                    rhs,
                    start=(ti == 0),
                    stop=(ti == NOFF - 1),
                )

        # ---- phase 1: q/k (fp8 DoubleRow, padded-coord tiles) + v (fp32r,
        # row-aligned tiles), norms, Gram. The two loops are interleaved as
        # priority hints; the Tile scheduler orders by dependencies.
        NSP = QK_ROWS * wp  # max padded span
        NQK = QK_ROWS * w  # max valid columns per q,k tile

        def qk_tile(i):
            rows = qk_rows[i]
            nspan = rows * wp
            nv = rows * w
            q_ps = ps_qk.tile([C, NSP], f32, tag="qk_ps")
            qk_mms_fp8(0, q_ps[:, :nspan], i, nspan)
            k_ps = ps_qk.tile([C, NSP], f32, tag="qk_ps")
            qk_mms_fp8(1, k_ps[:, :nspan], i, nspan)

            # compact valid columns (pad positions dropped) to bf16
            q_s = sb_qk.tile([C, NQK], bf16, tag="qk")
            nc.vector.tensor_copy(
                q_s[:, :nv].rearrange("p (r c) -> p r c", c=w),
                q_ps[:].rearrange("p (r c) -> p r c", c=wp)[
                    :, :rows, PAD : PAD + w
                ],
            )
            k_s = sb_qk.tile([C, NQK], bf16, tag="qk")
            nc.vector.tensor_copy(
                k_s[:, :nv].rearrange("p (r c) -> p r c", c=w),
                k_ps[:].rearrange("p (r c) -> p r c", c=wp)[
                    :, :rows, PAD : PAD + w
                ],
            )

            # norms: ACT square with accumulate on the compacted bf16 copies
            sq_q = sb_sq.tile([C, NQK], f32, tag="sq")
            nc.scalar.activation(
                sq_q[:, :nv], q_s[:, :nv], mybir.ActivationFunctionType.Square,
                accum_out=nq_p[:, i : i + 1],
            )
            sq_k = sb_sq.tile([C, NQK], f32, tag="sq")
            nc.scalar.activation(
                sq_k[:, :nv], k_s[:, :nv], mybir.ActivationFunctionType.Square,
                accum_out=nk_p[:, i : i + 1],
            )

            if dbg:
                nc.sync.dma_start(
                    dbg_d["dq"].ap()[:, i * NQK : i * NQK + nv], q_s[:, :nv]
                )
                nc.sync.dma_start(
                    dbg_d["dk"].ap()[:, i * NQK : i * NQK + nv], k_s[:, :nv]
                )

            # transpose 128-chunks; Gram accumulates G += q_chunk @ k_chunk^T
            qT = sb_qkT.tile([C, NQK], bf16, tag="qkT")
            kT = sb_qkT.tile([C, NQK], bf16, tag="qkT")
            for j in range(rows):
                t_ps = ps_tr.tile([C, C], bf16, tag="tr")
                nc.tensor.transpose(
                    t_ps[:], q_s[:, bass.ts(j, C)], ident_b[:]
                )
                nc.vector.tensor_copy(qT[:, bass.ts(j, C)], t_ps[:])
                t_ps2 = ps_tr.tile([C, C], bf16, tag="tr")
                nc.tensor.transpose(
                    t_ps2[:], k_s[:, bass.ts(j, C)], ident_b[:]
                )
                nc.vector.tensor_copy(kT[:, bass.ts(j, C)], t_ps2[:])
            for j in range(rows):
                nc.tensor.matmul(
                    g_ps[:],
                    qT[:, bass.ts(j, C)],
                    kT[:, bass.ts(j, C)],
                    start=(i == 0 and j == 0),
                    stop=(i == nt_qk - 1 and j == rows - 1),
                )

        for i in range(nt_qk):
            qk_tile(i)

        # ---- finale: softmax attention + fold with W_out ----
        nq = sb_f.tile([C, 1], f32, tag="nq")
        nc.vector.reduce_sum(nq[:], nq_p[:], axis=mybir.AxisListType.X)
        nk = sb_f.tile([C, 1], f32, tag="nk")
        nc.vector.reduce_sum(nk[:], nk_p[:], axis=mybir.AxisListType.X)
        # 1/||q|| = reciprocal(sqrt(sum q^2)); norms >> eps=1e-12 here
        nq_s = sb_f.tile([C, 1], f32, tag="nq_s")
        nc.scalar.sqrt(nq_s[:], nq[:])
        rq = sb_f.tile([C, 1], f32, tag="rq")
        nc.vector.reciprocal(rq[:], nq_s[:])
        nk_s = sb_f.tile([C, 1], f32, tag="nk_s")
        nc.scalar.sqrt(nk_s[:], nk[:])
        rk = sb_f.tile([C, 1], f32, tag="rk")
        nc.vector.reciprocal(rk[:], nk_s[:])
        # rq2 = rq * temperature(per-channel)
        rq2 = sb_f.tile([C, 1], f32, tag="rq2")
        nc.vector.tensor_mul(rq2[:], rq[:], tempc[:])

        # rk as a row, broadcast down partitions via outer product with ones
        rk_row_ps = ps_tr.tile([C, C], f32, tag="tr")
        nc.tensor.transpose(rk_row_ps[0:1, :], rk[:], ident[:])
        rk_row = sb_f.tile([1, C], f32, tag="rk_row")
        nc.vector.tensor_copy(rk_row[:], rk_row_ps[0:1, :])
        rkb_ps = ps_tr.tile([C, C], f32, tag="tr")
        nc.tensor.matmul(rkb_ps[:], ones1[:], rk_row[:], start=True, stop=True)
        rkb = sb_f.tile([C, C], f32, tag="rkb")
        nc.vector.tensor_copy(rkb[:], rkb_ps[:])

        # masked softmax over the full [C, C] Gram: off-head-block entries
        # get a -1e4 bias -> exp underflows to exactly 0, so the softmax
        # result IS the block-diagonal attention matrix A.
        g_s = sb_f.tile([C, C], f32, tag="g_s")
        nc.vector.tensor_copy(g_s[:], g_ps[:])
        g1 = sb_f.tile([C, C], f32, tag="g1")
        nc.vector.tensor_mul(g1[:], g_s[:], rkb[:])
        g2 = sb_f.tile([C, C], f32, tag="g2")
        nc.vector.tensor_scalar_mul(g2[:], g1[:], rq2[:])
        g3 = sb_f.tile([C, C], f32, tag="g3")
        nc.vector.tensor_add(g3[:], g2[:], maskn[:])
        mx = sb_f.tile([C, 1], f32, tag="mx")
        nc.vector.reduce_max(mx[:], g3[:], axis=mybir.AxisListType.X)
        nmx = sb_f.tile([C, 1], f32, tag="nmx")
        nc.vector.tensor_scalar_mul(nmx[:], mx[:], -1.0)
        ex = sb_f.tile([C, C], f32, tag="ex")
        ssum = sb_f.tile([C, 1], f32, tag="ssum")
        nc.scalar.activation(
            ex[:], g3[:], mybir.ActivationFunctionType.Exp,
            bias=nmx[:], accum_out=ssum[:],
        )
        rs = sb_f.tile([C, 1], f32, tag="rs")
        nc.vector.reciprocal(rs[:], ssum[:])
        a_bd = sb_f.tile([C, C], f32, tag="a_bd")
        nc.vector.tensor_scalar_mul(a_bd[:], ex[:], rs[:])
        # HAM keep-warm: touch finale intermediates with cheap PE transposes
        for warm in (g1, g3, ex, a_bd):
            wm_ps = ps_tr.tile([C, C], f32, tag="tr")
            nc.tensor.transpose(wm_ps[:], warm[:], ident[:])

        if dbg:
            nc.sync.dma_start(dbg_d["dg"].ap(), g_s[:])
            nc.sync.dma_start(dbg_d["dabd"].ap(), a_bd[:])
            nc.sync.dma_start(dbg_d["dnq"].ap(), nq[:])

        # M_final = (W_out A)^T = A^T W_out^T
        mf_ps = ps_tr.tile([C, C], f32, tag="tr")
        nc.tensor.matmul(mf_ps[:], a_bd[:], woutT[:], start=True, stop=True)
        m_final = sb_f.tile([C, C], f32, tag="m_final")
        nc.vector.tensor_copy(m_final[:].bitcast(f32r), mf_ps[:])
        if dbg:
            nc.sync.dma_start(dbg_d["dmf"].ap(), m_final[:])

        # ---- phase 2: fold M = W_out A into the v-conv weights, then the
        # out conv produces y directly. wB holds NON-transposed Bv_t blocks,
        # so matmul(lhsT=Bv_t, rhs=M^T) = Bv_t^T M^T = (M Bv_t)^T = conv lhsT.
        w_fold = sb_wf.tile([C, NOFF * C], bf16, tag="w_fold")
        for ti in range(NOFF):
            wf_ps = ps_tr.tile([C, C], f32, tag="tr")
            nc.tensor.matmul(
                wf_ps[:],
                wB[:, bass.ts(ti, C)].bitcast(f32r),
                m_final[:].bitcast(f32r),
                start=True,
                stop=True,
            )
            nc.vector.tensor_copy(w_fold[:, bass.ts(ti, C)], wf_ps[:])

        for i in range(nt):
            o_ps = ps_out.tile([C, N], f32, tag="out_ps")
            out_mms(o_ps, w_fold, i)
            o_s = sb_o.tile([C, N], f32, tag="o_s")
            nc.vector.tensor_copy(o_s[:], o_ps[:])
            nc.sync.dma_start(y_d.ap()[:, bass.ts(i, N)], o_s[:])

    nc.compile()
    return nc


def _pow2_scale(m, cap=224.0):
    return 2.0 ** np.floor(np.log2(cap / max(np.abs(m).max(), 1e-30)))


def _prep_inputs(inputs, h=H, w=W):
    Bm = fold_weights(
        inputs["w_qkv"], inputs["w_dw3"], inputs["w_dw5"], inputs["w_dw7"],
        inputs["w_q"], inputs["w_k"], inputs["w_v"],
    )
    offsets = [(dy, dx) for dy in range(-3, 4) for dx in range(-3, 4)]
    off_idx = {t: i for i, t in enumerate(offsets)}
    # v-conv fold operands: block t = B[2,t] (NOT transposed; the on-device
    # fold matmul(lhsT=Bv_t, rhs=M^T) produces the conv lhsT (M Bv_t)^T)
    wB = np.concatenate(
        [np.ascontiguousarray(Bm[2, ti], dtype=np.float32) for ti in range(NOFF)],
        axis=1,
    )
    # q,k fp8 weights in TAP_ORDER, scaled per-output (scale cancels in the
    # normalized attention, so no compensation is needed downstream)
    wb8_blocks = []
    for o in (0, 1):
        s = _pow2_scale(Bm[o])
        for dy, dx in TAP_ORDER:
            blk = (Bm[o, off_idx[(dy, dx)]].T * s).astype(np.float32)
            wb8_blocks.append(blk.astype(ml_dtypes.float8_e4m3))
    wb8 = np.concatenate(wb8_blocks, axis=1)  # [C, 2*49*C] fp8

    woutT = np.ascontiguousarray(np.asarray(inputs["w_out"]).T).astype(np.float32)
    tempc = np.repeat(
        np.asarray(inputs["temperature"], np.float32).reshape(HEADS), DH
    ).reshape(C, 1)
    ident = np.eye(C, dtype=np.float32)
    maskn = np.full((C, C), -1e4, np.float32)
    for hd in range(HEADS):
        maskn[hd * DH : (hd + 1) * DH, hd * DH : (hd + 1) * DH] = 0.0
    zcon = np.ones((C, C), np.float32)
    x = np.asarray(inputs["x"], np.float32)
    nb = x.shape[0]
    hp, wp = h + 2 * PAD, w + 2 * PAD
    xpad = np.zeros((nb, C, hp, wp), np.float32)
    xpad[:, :, PAD : PAD + h, PAD : PAD + w] = x.reshape(nb, C, h, w)
    xpad8 = np.zeros((nb, C, FRONT + hp * wp + SLACK), np.float32)
    xpad8[:, :, FRONT : FRONT + hp * wp] = xpad.reshape(nb, C, hp * wp)
    xpad8 = xpad8.astype(ml_dtypes.float8_e4m3)
    in_maps = [
        {
            "x": np.ascontiguousarray(
                xpad[b].reshape(C, hp * wp).astype(ml_dtypes.bfloat16)
            ),
            "x8": np.ascontiguousarray(xpad8[b]),
            "wB": wB,
            "wb8": wb8,
            "woutT": woutT,
            "tempc": tempc,
            "ident": ident,
            "maskn": maskn,
            "zcon": zcon,
        }
        for b in range(nb)
    ]
    return in_maps


def kernel(**inputs):
    if "nc" not in _NC_CACHE:
        _NC_CACHE["nc"] = build_nc()
    nc = _NC_CACHE["nc"]
    in_maps = _prep_inputs(inputs)
    res = run_bass_kernel_spmd(nc, in_maps, core_ids=list(range(B)))
    out = np.stack([res.results[b]["y"].reshape(C, H, W) for b in range(B)])
    return out.astype(np.float32)
